# revision 31
# baseline (speedup 1.0000x reference)
"""Pairwise IoU kernel for Trainium2 (8 NeuronCores, SPMD data-parallel).

anchor [1048576, 4] x target [64, 4] -> iou [1048576, 64]  (all float32)

Sharding: anchor rows split evenly across the 8 cores (131072 rows each);
target is replicated. Each core computes its [131072, 64] block of the
output independently; no collectives.

Written in raw Bass (not Tile): this toolchain's codegen accepts at most
one semaphore wait per instruction, so cross-engine dependencies are
expressed as standalone wait_ge instructions with hand-computed
cumulative thresholds. (Tile's auto-generated multi-wait sync does not
compile here; custom-DVE ops and fp32 TensorE matmuls are also
unusable on this toolchain/hardware combination.)

Per-core structure:
- one DMA loads all anchors ([128, T*M*4] = 16KB/partition; partition p
  holds anchor rows [p*T*M, (p+1)*T*M)); per-anchor areas precomputed in
  three whole-shard vector ops; target coords repacked to stride-1 rows
  (inner-strided reads cost ~2x on the vector engine)
- T=16 iterations over [128, M=64, K=64] blocks (free dim 4096) using
  stride-0 broadcast access patterns:
    DVE: min/max per axis (f32: the coordinate subtraction that follows
         is cancellation-sensitive), dx/dy subtractions written to bf16
         (relative error only), inter = dx*dy at bf16 2x mode,
         union via STT (1x) + bf16 add (2x)
    ACT: relu x2 in-place, then 1/union = Exp(-Ln(union)); Ln output
         kept in f32 (bf16 on a logarithm amplifies into ~4% error)
    DVE: iou = inter * recip at bf16 2x, emitted one iteration late
    ACT: casts the bf16 iou tile to the f32 store tile (ACT has slack;
         a gpsimd cast-DMA store was slower)
- contiguous operands use flattened [128, 4096] access patterns (2D
  [m, k] patterns pay ~11 cycles per inner row on this hardware)
- 16 output DMAs of 1MB each on the sync-engine HWDGE queue

Measured: 666 us on hardware (8 cores), rel err 4.7e-3 vs the f32
reference (bf16 intermediates; exact-f32 variant runs 876 us at 1e-5).
"""

import numpy as np

import concourse.bass as bass
import concourse.mybir as mybir
from concourse.bass_utils import run_bass_kernel_spmd

N = 1048576
K = 64
N_CORES = 8
NS = N // N_CORES  # 131072 anchors per core
P = 128
M = 64  # anchors per partition per iteration
F = M * K  # 4096 free elements per main op
T = NS // (P * M)  # 16 iterations
G = 1  # iterations per store group
S = T // G  # 8 stores
NA = T * M  # anchors per partition
DT = mybir.dt.float32
Alu = mybir.AluOpType
Act = mybir.ActivationFunctionType


def build_kernel_body(nc, ctx, anchor, target, out):
    a_r = anchor.rearrange("(p n) c -> p (n c)", p=P)  # [128, NA*4]
    o_r = out.rearrange("(p s gm) k -> p s (gm k)", p=P, gm=G * M)  # [P, S, G*F]

    def sb(shape, name, dt=DT):
        h = ctx.enter_context(nc.sbuf_tensor(name, shape, dt))
        return h[tuple(slice(None) for _ in shape)]  # handle -> full AP

    a_all = sb([P, NA * 4], "a_all")
    ttile = sb([P, K * 4], "ttile")
    tarea = sb([P, 1, K], "tarea", mybir.dt.bfloat16)
    ttmp = sb([P, 1, K], "ttmp")
    area_all = sb([P, NA, 1], "area_all")
    artmp = sb([P, NA, 1], "artmp")
    tA = sb([P, M, K], "tA")
    tMx = sb([P, M, K], "tMx")
    BF = mybir.dt.bfloat16
    tD = sb([P, M, K], "tD", BF)
    tD2 = sb([P, M, K], "tD2", BF)
    tI = sb([P, M, K], "tI", BF)
    tV = [sb([P, M, K], f"tV{i}", mybir.dt.bfloat16) for i in range(2)]
    tBb = [sb([P, G * F], f"tBb{i}", mybir.dt.bfloat16) for i in range(2)]
    tBf = [sb([P, G * F], f"tBf{i}") for i in range(2)]
    tL = [sb([P, M, K], f"tL{i}") for i in range(2)]

    tpack = sb([P, 4, K], "tpack")  # coordinate-major, stride-1 k rows
    av = a_all.rearrange("p (n c) -> p n c", c=4)
    tv = ttile.rearrange("p (k c) -> p c k", c=4)
    tx1, ty1, tx2, ty2 = (tpack[:, c : c + 1, :] for c in range(4))
    bc = (P, M, K)

    dma_sem = ctx.enter_context(nc.semaphore("dma_sem"))
    st_sem = ctx.enter_context(nc.semaphore("st_sem"))
    dve_sem = ctx.enter_context(nc.semaphore("dve_sem"))
    act_sem = ctx.enter_context(nc.semaphore("act_sem"))

    # --- per-iteration semaphore tick schedules (cumulative counts) ---
    # DVE ops: prolog 6; per iter: 6 minmax/sub + (iou if t>=1) + inter + 2 union
    # ACT ops per iter: relu_x, relu_y, Ln, Exp
    dve_subx = {}
    dve_suby = {}
    dve_uadd = {}
    dve_iou = {}
    act_reluy = {}
    act_exp = {}
    act_cast = {}
    dve_n = 10  # prolog: 4 target-pack copies + 6 area ops
    act_n = 0
    for t in range(T):
        dve_subx[t] = dve_n + 3
        dve_suby[t] = dve_n + 6
        n_iou = 1 if t >= 1 else 0
        if t >= 1:
            dve_iou[t - 1] = dve_n + 6 + n_iou
        dve_uadd[t] = dve_n + 6 + n_iou + 3  # after inter + ustt + uadd
        dve_n = dve_uadd[t]
        act_reluy[t] = act_n + 2
        act_exp[t] = act_n + 4
        if t == 0:
            act_n += 4
        else:
            act_cast[t - 1] = act_n + 5  # appended after Exp(t)
            act_n += 5
    dve_iou[T - 1] = dve_n + 1
    dve_n += 1
    act_cast[T - 1] = act_n + 1

    block = ctx.enter_context(nc.Block())

    @block.gpsimd
    def _(g):
        g.dma_start(out=a_all, in_=a_r).then_inc(dma_sem, 16)
        g.dma_start(
            out=ttile,
            in_=target.rearrange("k c -> (k c)")[None].broadcast_to((P, K * 4)),
        ).then_inc(dma_sem, 16)

    @block.vector
    def _(v):
        def tt(out, in0, in1, op):
            nc.vector.tensor_tensor(out=out, in0=in0, in1=in1, op=op).then_inc(
                dve_sem, 1
            )

        v.wait_ge(dma_sem, 32)
        # pack target coords to stride-1 rows (strided reads are ~2x slower)
        for c in range(4):
            nc.vector.tensor_copy(
                out=tpack[:, c : c + 1, :], in_=tv[:, c : c + 1, :]
            ).then_inc(dve_sem, 1)
        # target area [P,1,K]
        tt(ttmp, tx2, tx1, Alu.subtract)
        tt(tarea, ty2, ty1, Alu.subtract)
        tt(tarea, tarea, ttmp, Alu.mult)
        # anchor area [P,NA,1]
        tt(area_all, av[:, :, 2:3], av[:, :, 0:1], Alu.subtract)
        tt(artmp, av[:, :, 3:4], av[:, :, 1:2], Alu.subtract)
        tt(area_all, area_all, artmp, Alu.mult)

        def emit_iou(pt):
            # iou(pt) = inter(pt) * recip(pt), bf16 2x into the cast tile
            v.wait_ge(act_sem, act_exp[pt])
            tt(tBb[pt % 2][:, :], tI.rearrange("p m k -> p (m k)"), tL[pt % 2].rearrange("p m k -> p (m k)"), Alu.mult)

        for t in range(T):
            slc = slice(t * M, (t + 1) * M)
            ax1 = av[:, slc, 0:1]
            ay1 = av[:, slc, 1:2]
            ax2 = av[:, slc, 2:3]
            ay2 = av[:, slc, 3:4]
            aa = area_all[:, slc, :]

            tt(tA, ax2.broadcast_to(bc), tx2.broadcast_to(bc), Alu.min)
            tt(tMx, ax1.broadcast_to(bc), tx1.broadcast_to(bc), Alu.max)
            tt(tD.rearrange("p m k -> p (m k)"), tA.rearrange("p m k -> p (m k)"), tMx.rearrange("p m k -> p (m k)"), Alu.subtract)
            tt(tA, ay2.broadcast_to(bc), ty2.broadcast_to(bc), Alu.min)
            tt(tMx, ay1.broadcast_to(bc), ty1.broadcast_to(bc), Alu.max)
            tt(tD2.rearrange("p m k -> p (m k)"), tA.rearrange("p m k -> p (m k)"), tMx.rearrange("p m k -> p (m k)"), Alu.subtract)
            if t >= 1:
                emit_iou(t - 1)
            v.wait_ge(act_sem, act_reluy[t])
            tt(tI.rearrange("p m k -> p (m k)"), tD.rearrange("p m k -> p (m k)"), tD2.rearrange("p m k -> p (m k)"), Alu.mult)  # inter = relu(dx)*relu(dy)
            # union = (aa - inter) + tarea
            nc.vector.scalar_tensor_tensor(
                out=tV[t % 2],
                in0=tI,
                scalar=-1.0,
                in1=aa.broadcast_to(bc),
                op0=Alu.mult,
                op1=Alu.add,
            ).then_inc(dve_sem, 1)
            tt(tV[t % 2], tV[t % 2], tarea.broadcast_to(bc), Alu.add)
        emit_iou(T - 1)


    @block.sync
    def _(sy):
        for s in range(S):
            sy.wait_ge(act_sem, act_cast[s])
            sy.dma_start(out=o_r[:, s, :], in_=tBf[s % 2]).then_inc(st_sem, 16)

    @block.scalar
    def _(a):
        for t in range(T):
            a.wait_ge(dve_sem, dve_subx[t])
            nc.scalar.activation(
                out=tD.rearrange("p m k -> p (m k)"), in_=tD.rearrange("p m k -> p (m k)"), func=Act.Relu
            ).then_inc(act_sem, 1)
            a.wait_ge(dve_sem, dve_suby[t])
            nc.scalar.activation(
                out=tD2.rearrange("p m k -> p (m k)"), in_=tD2.rearrange("p m k -> p (m k)"), func=Act.Relu
            ).then_inc(act_sem, 1)
            a.wait_ge(dve_sem, dve_uadd[t])
            nc.scalar.activation(
                out=tL[t % 2].rearrange("p m k -> p (m k)"), in_=tV[t % 2].rearrange("p m k -> p (m k)"), func=Act.Ln
            ).then_inc(act_sem, 1)
            nc.scalar.activation(
                out=tL[t % 2].rearrange("p m k -> p (m k)"), in_=tL[t % 2].rearrange("p m k -> p (m k)"), func=Act.Exp, scale=-1.0
            ).then_inc(act_sem, 1)
            if t >= 1:
                if t - 1 >= 2:
                    a.wait_ge(st_sem, 16 * (t - 2))
                a.wait_ge(dve_sem, dve_iou[t - 1])
                nc.scalar.activation(
                    out=tBf[(t - 1) % 2][:, :], in_=tBb[(t - 1) % 2][:, :], func=Act.Copy
                ).then_inc(act_sem, 1)
        a.wait_ge(st_sem, 16 * (T - 2))
        a.wait_ge(dve_sem, dve_iou[T - 1])
        nc.scalar.activation(
            out=tBf[(T - 1) % 2][:, :], in_=tBb[(T - 1) % 2][:, :], func=Act.Copy
        ).then_inc(act_sem, 1)



_NC_CACHE = {}


def build_nc():
    if "nc" in _NC_CACHE:
        return _NC_CACHE["nc"]
    from contextlib import ExitStack

    nc = bass.Bass()
    anchor = nc.declare_dram_parameter("anchor", [NS, 4], DT, isOutput=False)
    target = nc.declare_dram_parameter("target", [K, 4], DT, isOutput=False)
    out = nc.declare_dram_parameter("out", [NS, K], DT, isOutput=True)
    with ExitStack() as ctx:
        build_kernel_body(nc, ctx, anchor, target, out)
    _NC_CACHE["nc"] = nc
    return nc


def kernel(anchor, target, _trace=False):
    nc = build_nc()
    anchor = np.ascontiguousarray(anchor, dtype=np.float32)
    target = np.ascontiguousarray(target, dtype=np.float32)
    in_maps = [
        {"anchor": np.ascontiguousarray(anchor[i * NS : (i + 1) * NS]), "target": target}
        for i in range(N_CORES)
    ]
    res = run_bass_kernel_spmd(
        nc, in_maps, core_ids=list(range(N_CORES)), trace=_trace
    )
    full = np.concatenate([r["out"] for r in res.results], axis=0)
    if _trace:
        return full, res
    return full
